# revision 1
# baseline (speedup 1.0000x reference)
"""Two-layer LSTM (batch=64, feature=256, seq=2048, hidden=16) + mean-pool
over hidden dim, on 8 Trainium2 NeuronCores.

Strategy
--------
Data-parallel over batch: each core gets 8 samples. Per core, the LSTM
recurrence is solved by Picard (Jacobi) iteration over the whole sequence:
the cell-state propagation c_t = sigmoid(f_t)*c_{t-1} + z_t is computed
EXACTLY each sweep with the DVE tensor_tensor_scan instruction, and only the
small h-feedback term (Whh @ h_{t-1}, spectral gain ~0.16 for layer 0 /
~0.3 for layer 1) is lagged one sweep. 10 / 12 sweeps reach the fp32 fixed
point (verified vs the serial reference in fp64: ~1e-7 rel).

Layout: partition p = b_local*16 + j (8 samples x 16 hidden channels = 128
partitions), free dim = time. Gate pre-activations are produced per gate
type tau in {i,f,g,o} as [128, 2048] tiles; the h-feedback GEMM uses a
block-diagonal lhsT = kron(I8, Whh_tau^T) so one K=128 matmul covers all 8
samples. The input GEMM runs in natural [64-gates, time] layout per sample
and is re-laid-out to [(b,j), tau*T+t] with 16-partition SBUF->SBUF DMAs.
"""

import numpy as np
from contextlib import ExitStack

B_TOT, D_IN, T_SEQ, H_DIM = 64, 256, 2048, 16
N_CORES = 8
B_LOC = B_TOT // N_CORES  # 8
G4 = 4 * H_DIM  # 64
K0_ITERS = 10
K1_ITERS = 12
NSUB = T_SEQ // 512  # matmul free-dim subtiles

_cache = {}


def _build_module():
    import concourse.bacc as bacc
    import concourse.mybir as mybir
    import concourse.tile as tile

    f32 = mybir.dt.float32
    AF = mybir.ActivationFunctionType
    ALU = mybir.AluOpType

    nc = bacc.Bacc("TRN2", target_bir_lowering=False, debug=False,
                   num_devices=N_CORES)

    x_d = nc.dram_tensor("x", (B_LOC, D_IN, T_SEQ), f32, kind="ExternalInput")
    w0_d = nc.dram_tensor("w0", (128, 128), f32, kind="ExternalInput")
    fb0_d = nc.dram_tensor("fb0", (128, 512), f32, kind="ExternalInput")
    xw1_d = nc.dram_tensor("xw1", (128, 512), f32, kind="ExternalInput")
    fb1_d = nc.dram_tensor("fb1", (128, 512), f32, kind="ExternalInput")
    id_d = nc.dram_tensor("ident", (128, 128), f32, kind="ExternalInput")
    b0_d = nc.dram_tensor("b0", (128, 4), f32, kind="ExternalInput")
    b1_d = nc.dram_tensor("b1", (128, 4), f32, kind="ExternalInput")
    mp_d = nc.dram_tensor("mp", (128, 8), f32, kind="ExternalInput")
    y_d = nc.dram_tensor("y", (B_LOC, T_SEQ), f32, kind="ExternalOutput")

    with tile.TileContext(nc) as tc, ExitStack() as ctx:
        cpool = ctx.enter_context(tc.tile_pool(name="consts", bufs=1))
        xpool = ctx.enter_context(tc.tile_pool(name="xstage", bufs=3))
        spool = ctx.enter_context(tc.tile_pool(name="stage", bufs=2))
        gpool = ctx.enter_context(tc.tile_pool(name="bigbufs", bufs=1))
        wpool = ctx.enter_context(tc.tile_pool(name="work", bufs=1))
        ppool = ctx.enter_context(
            tc.tile_pool(name="psum", bufs=2, space="PSUM"))

        def const(name, dram):
            t = cpool.tile(list(dram.shape), f32, name=name)
            nc.sync.dma_start(t[:], dram.ap())
            return t

        w0_sb = const("w0_sb", w0_d)
        fb0_sb = const("fb0_sb", fb0_d)
        xw1_sb = const("xw1_sb", xw1_d)
        fb1_sb = const("fb1_sb", fb1_d)
        id_sb = const("id_sb", id_d)
        b0_sb = const("b0_sb", b0_d)
        b1_sb = const("b1_sb", b1_d)
        mp_sb = const("mp_sb", mp_d)

        xg = gpool.tile([128, 4 * T_SEQ], f32, name="xg")
        h0 = gpool.tile([128, T_SEQ + 1], f32, name="h0")
        h1 = gpool.tile([128, T_SEQ + 1], f32, name="h1")

        # ---- Phase A: layer-0 input GEMM + relayout --------------------
        # Per sample pair: gates psum [128=(2 samples x 64 gates), T].
        for pair in range(4):
            xa = xpool.tile([128, 2 * T_SEQ], f32, name="xa", tag="xst")
            xb = xpool.tile([128, 2 * T_SEQ], f32, name="xb", tag="xst")
            for bl in range(2):
                b = pair * 2 + bl
                nc.sync.dma_start(xa[:, bl * T_SEQ:(bl + 1) * T_SEQ],
                                  x_d.ap()[b, 0:128, :])
                nc.sync.dma_start(xb[:, bl * T_SEQ:(bl + 1) * T_SEQ],
                                  x_d.ap()[b, 128:256, :])
            ps = ppool.tile([128, T_SEQ], f32, name="psA", tag="mm")
            for bl in range(2):
                for n in range(NSUB):
                    osl = slice(n * 512, (n + 1) * 512)
                    isl = slice(bl * T_SEQ + n * 512, bl * T_SEQ + (n + 1) * 512)
                    nc.tensor.matmul(ps[bl * 64:(bl + 1) * 64, osl],
                                     w0_sb[:, 0:64], xa[:, isl],
                                     start=True, stop=False)
                    nc.tensor.matmul(ps[bl * 64:(bl + 1) * 64, osl],
                                     w0_sb[:, 64:128], xb[:, isl],
                                     start=False, stop=True)
            st = spool.tile([128, T_SEQ], f32, name="st", tag="stage")
            if pair % 2 == 0:
                nc.scalar.copy(st[:], ps[:])
            else:
                nc.vector.tensor_copy(st[:], ps[:])
            for bl in range(2):
                b = pair * 2 + bl
                for tau in range(4):
                    nc.sync.dma_start(
                        xg[b * 16:(b + 1) * 16, tau * T_SEQ:(tau + 1) * T_SEQ],
                        st[bl * 64 + tau * 16:bl * 64 + (tau + 1) * 16, :])

        # ---- Shared per-layer Picard sweep -----------------------------
        def run_layer(h, fb_sb, bias_sb, iters):
            nc.vector.memset(h[:, 0:1], 0.0)
            TAUS = (0, 2, 1, 3)  # i, g, f, o: z needs (i,g), scan f, then o
            for it in range(iters):
                acts = {}
                for tau in TAUS:
                    func = AF.Tanh if tau == 2 else AF.Sigmoid
                    dst = wpool.tile([128, T_SEQ], f32, name=f"act{tau}",
                                     tag=f"act{tau}")
                    if it == 0:
                        nc.scalar.activation(
                            dst[:], xg[:, tau * T_SEQ:(tau + 1) * T_SEQ],
                            func, bias=bias_sb[:, tau:tau + 1])
                    else:
                        ps = ppool.tile([128, T_SEQ], f32, name="psI", tag="mm")
                        for n in range(NSUB):
                            osl = slice(n * 512, (n + 1) * 512)
                            nc.tensor.matmul(
                                ps[:, osl], id_sb[:],
                                xg[:, tau * T_SEQ + n * 512:
                                   tau * T_SEQ + (n + 1) * 512],
                                start=True, stop=False)
                            nc.tensor.matmul(
                                ps[:, osl], fb_sb[:, tau * 128:(tau + 1) * 128],
                                h[:, n * 512:(n + 1) * 512],
                                start=False, stop=True)
                        nc.scalar.activation(dst[:], ps[:], func,
                                             bias=bias_sb[:, tau:tau + 1])
                    acts[tau] = dst
                z = wpool.tile([128, T_SEQ], f32, name="z", tag="z")
                nc.vector.tensor_mul(z[:], acts[0][:], acts[2][:])
                cs = wpool.tile([128, T_SEQ], f32, name="cs", tag="cs")
                nc.vector.tensor_tensor_scan(cs[:], acts[1][:], z[:], 0.0,
                                             ALU.mult, ALU.add)
                tct = wpool.tile([128, T_SEQ], f32, name="tct", tag="tct")
                nc.scalar.activation(tct[:], cs[:], AF.Tanh)
                nc.vector.tensor_mul(h[:, 1:T_SEQ + 1], acts[3][:], tct[:])

        # ---- Phase B: layer-0 sweeps -----------------------------------
        run_layer(h0, fb0_sb, b0_sb, K0_ITERS)

        # ---- Phase C: layer-1 input GEMM (block-diag, in-layout) -------
        for tau in range(4):
            ps = ppool.tile([128, T_SEQ], f32, name="psC", tag="mm")
            for n in range(NSUB):
                nc.tensor.matmul(ps[:, n * 512:(n + 1) * 512],
                                 xw1_sb[:, tau * 128:(tau + 1) * 128],
                                 h0[:, 1 + n * 512:1 + (n + 1) * 512],
                                 start=True, stop=True)
            if tau % 2 == 0:
                nc.scalar.copy(xg[:, tau * T_SEQ:(tau + 1) * T_SEQ], ps[:])
            else:
                nc.vector.tensor_copy(xg[:, tau * T_SEQ:(tau + 1) * T_SEQ],
                                      ps[:])

        # ---- Phase D: layer-1 sweeps -----------------------------------
        run_layer(h1, fb1_sb, b1_sb, K1_ITERS)

        # ---- Phase E: mean over hidden dim + store ---------------------
        psE = ppool.tile([8, T_SEQ], f32, name="psE", tag="mm")
        for n in range(NSUB):
            nc.tensor.matmul(psE[:, n * 512:(n + 1) * 512], mp_sb[:],
                             h1[:, 1 + n * 512:1 + (n + 1) * 512],
                             start=True, stop=True)
        yt = wpool.tile([8, T_SEQ], f32, name="yt", tag="yt")
        nc.scalar.copy(yt[:], psE[:])
        nc.sync.dma_start(y_d.ap(), yt[:])

    nc.compile()
    return nc


def _pack_weights(Wih0, Whh0, bih0, bhh0, Wih1, Whh1, bih1, bhh1):
    I8 = np.eye(8, dtype=np.float32)
    w0 = np.zeros((128, 128), np.float32)
    w0[:, 0:64] = Wih0.T[0:128]
    w0[:, 64:128] = Wih0.T[128:256]

    def blk(W):  # (64,16) -> (128,512); tau slice = kron(I8, W_tau^T)
        out = np.zeros((128, 512), np.float32)
        for tau in range(4):
            out[:, tau * 128:(tau + 1) * 128] = np.kron(
                I8, W[tau * 16:(tau + 1) * 16].T)
        return out

    def bv(bi, bh):
        b = (bi + bh).astype(np.float32)
        out = np.zeros((128, 4), np.float32)
        for tau in range(4):
            out[:, tau] = np.tile(b[tau * 16:(tau + 1) * 16], 8)
        return out

    return {
        "w0": w0,
        "fb0": blk(Whh0),
        "xw1": blk(Wih1),
        "fb1": blk(Whh1),
        "ident": np.eye(128, dtype=np.float32),
        "b0": bv(bih0, bhh0),
        "b1": bv(bih1, bhh1),
        "mp": np.kron(I8, np.ones((16, 1), np.float32) / 16.0),
    }


def kernel(x, Wih0, Whh0, bih0, bhh0, Wih1, Whh1, bih1, bhh1, _trace=False):
    from concourse import bass_utils

    x = np.ascontiguousarray(np.asarray(x, dtype=np.float32))
    consts = _pack_weights(
        np.asarray(Wih0, np.float32), np.asarray(Whh0, np.float32),
        np.asarray(bih0, np.float32), np.asarray(bhh0, np.float32),
        np.asarray(Wih1, np.float32), np.asarray(Whh1, np.float32),
        np.asarray(bih1, np.float32), np.asarray(bhh1, np.float32))

    if "nc" not in _cache:
        _cache["nc"] = _build_module()
    nc = _cache["nc"]

    in_maps = []
    for c in range(N_CORES):
        m = {"x": np.ascontiguousarray(x[c * B_LOC:(c + 1) * B_LOC])}
        m.update(consts)
        in_maps.append(m)

    res = bass_utils.run_bass_kernel_spmd(
        nc, in_maps, core_ids=list(range(N_CORES)), trace=_trace)
    y = np.concatenate([r["y"] for r in res.results], axis=0)
    if _trace:
        _cache["last_results"] = res
    return y


# revision 30
# speedup vs baseline: 13066.5022x; 13066.5022x over previous
"""Two-layer LSTM (batch=64, feature=256, seq=2048, hidden=16) + mean-pool
over hidden dim, on 8 Trainium2 NeuronCores.

Strategy
--------
Data-parallel over batch: each core gets 8 samples. Per core, the LSTM
recurrence is solved by Picard (Jacobi) iteration over the whole sequence
instead of a 2048-step serial loop: the cell-state propagation
c_t = sigmoid(f_t)*c_{t-1} + z_t is computed exactly each sweep with the
DVE tensor_tensor_scan instruction, and only the small h-feedback term
(Whh @ h_{t-1}, spectral gain ~0.16 for layer 0 / ~0.3 for layer 1) is
lagged one sweep. Each sweep is fully parallel over time, so it runs as a
handful of [128, 2048] tile ops. 5 / 7 sweeps converge to ~2e-4 relative
(the floor set by fp32r matmul rounding; more sweeps reach ~1.6e-4).

Layout: partition p = b_local*16 + j (8 samples x 16 hidden channels = 128
partitions), free dim = time. Gate pre-activations are produced per gate
type tau in {i,f,g,o} as [128, 2048] tiles; the h-feedback GEMM uses a
block-diagonal lhsT = kron(I8, Whh_tau^T) so one K=128 matmul covers all 8
samples. Layer 1 needs no materialized input GEMM: its per-sweep "input"
matmul is blockdiag(Wih1) @ h0 directly. The layer-0 input GEMM runs in
natural [64-gates, time] layout per sample pair and is re-laid-out to
[(b,j), tau*T+t] with 16-partition SBUF->SBUF DMAs on the GPSIMD DGE
queue (so it doesn't stall the x input stream on the SP queue).

Precision: sweep matmuls use fp32r (1 PE cycle/row vs 4 for fp32; TF32-ish
~1.6e-4 operand rounding, verified on HW). The layer-0 input GEMM uses a
host-side bf16 hi/lo split of x and Wih0 with three accumulated bf16
matmuls (~2e-5 accurate, and bf16 matmuls have no fp32r dst-partition-0
restriction, which the per-pair [128=2 samples x 64 gates, T] psum layout
would violate). Gate biases ride the ACT activation bias port.
"""

import numpy as np
from contextlib import ExitStack

B_TOT, D_IN, T_SEQ, H_DIM = 64, 256, 2048, 16
N_CORES = 8
B_LOC = B_TOT // N_CORES  # 8
G4 = 4 * H_DIM  # 64
K0_ITERS = 5
K1_ITERS = 7
NSUB = T_SEQ // 512  # matmul free-dim subtiles
USE_F32R = True  # fp32 "replicated" matmul mode: 1 cycle/row vs 4 for fp32
CHUNK_TAIL = 2  # split z/scan/tanh(c)/h ops into this many time chunks
H_ON_POOL = False  # run the h = sig_o * tanh(c) multiply on GPSIMD
TAU_ORDER = (0, 2, 1, 3)  # ACT processing order of gates i,g,f,o
PSUM_HALF = True  # psum tiles [128,1024] bufs=4 + per-half sigma
ACT_BUFS = 1

_cache = {}


def _build_module():
    import concourse.bacc as bacc
    import concourse.mybir as mybir
    import concourse.tile as tile

    f32 = mybir.dt.float32
    f32r = mybir.dt.float32r if USE_F32R else f32
    AF = mybir.ActivationFunctionType
    ALU = mybir.AluOpType

    nc = bacc.Bacc("TRN2", target_bir_lowering=False, debug=False,
                   num_devices=N_CORES)

    def mm(out, lhsT, rhs, start, stop):
        nc.tensor.matmul(out, lhsT, rhs, start=start, stop=stop)

    bf16 = mybir.dt.bfloat16
    xh_d = nc.dram_tensor("xh", (B_LOC, D_IN, T_SEQ), bf16,
                          kind="ExternalInput")
    xl_d = nc.dram_tensor("xl", (B_LOC, D_IN, T_SEQ), bf16,
                          kind="ExternalInput")
    w0h_d = nc.dram_tensor("w0h", (128, 128), bf16, kind="ExternalInput")
    w0l_d = nc.dram_tensor("w0l", (128, 128), bf16, kind="ExternalInput")
    fb0_d = nc.dram_tensor("fb0", (128, 512), f32r, kind="ExternalInput")
    xw1_d = nc.dram_tensor("xw1", (128, 512), f32r, kind="ExternalInput")
    fb1_d = nc.dram_tensor("fb1", (128, 512), f32r, kind="ExternalInput")
    id_d = nc.dram_tensor("ident", (128, 128), f32r, kind="ExternalInput")
    b0_d = nc.dram_tensor("b0", (128, 4), f32, kind="ExternalInput")
    b1_d = nc.dram_tensor("b1", (128, 4), f32, kind="ExternalInput")
    mp_d = nc.dram_tensor("mp", (128, 8), f32r, kind="ExternalInput")
    y_d = nc.dram_tensor("y", (B_LOC, T_SEQ), f32, kind="ExternalOutput")

    with tile.TileContext(nc) as tc, ExitStack() as ctx:
        cpool = ctx.enter_context(tc.tile_pool(name="consts", bufs=1))
        xpool = ctx.enter_context(tc.tile_pool(name="xstage", bufs=8))
        spool = ctx.enter_context(tc.tile_pool(name="stage", bufs=3))
        gpool = ctx.enter_context(tc.tile_pool(name="bigbufs", bufs=1))
        wpool = ctx.enter_context(tc.tile_pool(name="work", bufs=1))
        ppool = ctx.enter_context(
            tc.tile_pool(name="psum", bufs=2, space="PSUM"))

        def const(name, dram):
            t = cpool.tile(list(dram.shape), dram.dtype, name=name)
            nc.sync.dma_start(t[:], dram.ap())
            return t

        w0h_sb = const("w0h_sb", w0h_d)
        w0l_sb = const("w0l_sb", w0l_d)
        fb0_sb = const("fb0_sb", fb0_d)
        xw1_sb = const("xw1_sb", xw1_d)
        fb1_sb = const("fb1_sb", fb1_d)
        id_sb = const("id_sb", id_d)
        b0_sb = const("b0_sb", b0_d)
        b1_sb = const("b1_sb", b1_d)
        mp_sb = const("mp_sb", mp_d)

        xg = gpool.tile([128, 4 * T_SEQ], f32r, name="xg")
        h0 = gpool.tile([128, T_SEQ + 1], f32r, name="h0")
        h1 = gpool.tile([128, T_SEQ + 1], f32r, name="h1")

        # ---- Phase A: layer-0 input GEMM + relayout --------------------
        # Per sample pair: gates psum [128=(2 samples x 64 gates), T].
        # x streamed per (pair, k-chunk) at 2MB grain so the DMA queue
        # stays saturated while PE/copies trail behind.
        for pair in range(4):
            xtiles = {}
            for bl in range(2):
                for k in range(2):
                    b = pair * 2 + bl
                    xth = xpool.tile([128, T_SEQ], bf16, name=f"xh{bl}{k}",
                                     tag="xst")
                    nc.sync.dma_start(xth[:],
                                      xh_d.ap()[b, k * 128:(k + 1) * 128, :])
                    xtl = xpool.tile([128, T_SEQ], bf16, name=f"xl{bl}{k}",
                                     tag="xst")
                    nc.sync.dma_start(xtl[:],
                                      xl_d.ap()[b, k * 128:(k + 1) * 128, :])
                    xtiles[(bl, k)] = (xth, xtl)
            st = spool.tile([128, T_SEQ], f32r, name="st", tag="stage")
            halves = 2 if PSUM_HALF else 1
            HWA = T_SEQ // halves
            for hh in range(halves):
                if PSUM_HALF:
                    ps = ppool.tile([128, HWA], f32, name="psA",
                                    tag="mmh", bufs=4)
                else:
                    ps = ppool.tile([128, T_SEQ], f32, name="psA", tag="mm")
                for bl in range(2):
                    for nn in range(HWA // 512):
                        n = hh * (HWA // 512) + nn
                        osl = slice(nn * 512, (nn + 1) * 512)
                        isl = slice(n * 512, (n + 1) * 512)
                        out = ps[bl * 64:(bl + 1) * 64, osl]
                        for k in range(2):
                            xth, xtl = xtiles[(bl, k)]
                            wh = w0h_sb[:, k * 64:(k + 1) * 64]
                            wl = w0l_sb[:, k * 64:(k + 1) * 64]
                            mm(out, wh, xth[:, isl], k == 0, False)
                            mm(out, wl, xth[:, isl], False, False)
                            mm(out, wh, xtl[:, isl], False, k == 1)
                dstsl = slice(hh * HWA, (hh + 1) * HWA)
                if pair % 2 == 0:
                    nc.scalar.copy(st[:, dstsl], ps[:])
                else:
                    nc.vector.tensor_copy(st[:, dstsl], ps[:])
            for bl in range(2):
                b = pair * 2 + bl
                for tau in range(4):
                    nc.gpsimd.dma_start(
                        xg[b * 16:(b + 1) * 16, tau * T_SEQ:(tau + 1) * T_SEQ],
                        st[bl * 64 + tau * 16:bl * 64 + (tau + 1) * 16, :])

        # ---- Shared per-layer Picard sweep -----------------------------
        def run_layer(h, in_lhsT, in_rhs, in_off, xg_direct, fb_sb,
                      bias_sb, iters):
            # gate pre-acts per sweep: psum_tau = in_lhsT_tau @ in_rhs
            #                                   + fb_tau @ h (it>0)
            # L0: in_lhsT=identity, in_rhs=xg (tau-blocked columns)
            # L1: in_lhsT=blockdiag(Wih1), in_rhs=h0 (no materialized xg1)
            nc.vector.memset(h[:, 0:1].bitcast(f32), 0.0)
            TAUS = TAU_ORDER  # gate processing order on ACT
            CH = T_SEQ // CHUNK_TAIL
            for it in range(iters):
                acts = {}
                for tau in TAUS:
                    func = AF.Tanh if tau == 2 else AF.Sigmoid
                    dst = wpool.tile([128, T_SEQ], f32, name=f"act{tau}",
                                     tag=f"act{tau}", bufs=ACT_BUFS)
                    if it == 0 and xg_direct:
                        nc.scalar.activation(
                            dst[:],
                            in_rhs[:, tau * T_SEQ:tau * T_SEQ + T_SEQ]
                            .bitcast(f32),
                            func, bias=bias_sb[:, tau:tau + 1])
                        acts[tau] = dst
                        continue
                    halves = 2 if PSUM_HALF else 1
                    HW_ = T_SEQ // halves
                    for hh in range(halves):
                        if PSUM_HALF:
                            ps = ppool.tile([128, HW_], f32, name="psI",
                                            tag="mmh", bufs=4)
                        else:
                            ps = ppool.tile([128, T_SEQ], f32, name="psI",
                                            tag="mm")
                        for nn in range(HW_ // 512):
                            n = hh * (HW_ // 512) + nn
                            osl = slice(nn * 512, (nn + 1) * 512)
                            if xg_direct:
                                rsl = slice(tau * T_SEQ + n * 512,
                                            tau * T_SEQ + (n + 1) * 512)
                                lhsT = in_lhsT
                            else:
                                rsl = slice(in_off + n * 512,
                                            in_off + (n + 1) * 512)
                                lhsT = in_lhsT[:, tau * 128:(tau + 1) * 128]
                            mm(ps[:, osl], lhsT, in_rhs[:, rsl],
                               True, it == 0)
                            if it > 0:
                                mm(ps[:, osl],
                                   fb_sb[:, tau * 128:(tau + 1) * 128],
                                   h[:, n * 512:(n + 1) * 512], False, True)
                        nc.scalar.activation(
                            dst[:, hh * HW_:(hh + 1) * HW_], ps[:], func,
                            bias=bias_sb[:, tau:tau + 1])
                    acts[tau] = dst
                cs_prev = None
                for n in range(CHUNK_TAIL):
                    sl = slice(n * CH, (n + 1) * CH)
                    z = wpool.tile([128, CH], f32, name="z", tag="z", bufs=2)
                    cs = wpool.tile([128, CH], f32, name="cs", tag="cs",
                                    bufs=2)
                    tct = wpool.tile([128, CH], f32, name="tct", tag="tct",
                                     bufs=2)
                    nc.vector.tensor_mul(z[:], acts[0][:, sl],
                                         acts[2][:, sl])
                    init = 0.0 if n == 0 else cs_prev[:, CH - 1:CH]
                    nc.vector.tensor_tensor_scan(cs[:], acts[1][:, sl],
                                                 z[:], init,
                                                 ALU.mult, ALU.add)
                    nc.scalar.activation(tct[:], cs[:], AF.Tanh)
                    eng = nc.gpsimd if H_ON_POOL else nc.vector
                    eng.tensor_mul(h[:, 1 + n * CH:1 + (n + 1) * CH],
                                   acts[3][:, sl], tct[:])
                    cs_prev = cs

        # ---- Phase B: layer-0 sweeps -----------------------------------
        run_layer(h0, id_sb, xg, 0, True, fb0_sb, b0_sb, K0_ITERS)

        # ---- Phase D: layer-1 sweeps -----------------------------------
        run_layer(h1, xw1_sb, h0, 1, False, fb1_sb, b1_sb, K1_ITERS)

        # ---- Phase E: mean over hidden dim + store ---------------------
        yt = spool.tile([8, T_SEQ], f32, name="yt", tag="stage")
        halves = 2 if PSUM_HALF else 1
        HWE = T_SEQ // halves
        for hh in range(halves):
            if PSUM_HALF:
                psE = ppool.tile([8, HWE], f32, name="psE", tag="mmh", bufs=4)
            else:
                psE = ppool.tile([8, T_SEQ], f32, name="psE", tag="mm")
            for nn in range(HWE // 512):
                n = hh * (HWE // 512) + nn
                mm(psE[:, nn * 512:(nn + 1) * 512], mp_sb[:],
                   h1[:, 1 + n * 512:1 + (n + 1) * 512], True, True)
            nc.scalar.copy(yt[:, hh * HWE:(hh + 1) * HWE], psE[:])
        nc.sync.dma_start(y_d.ap(), yt[:])

    nc.compile()
    return nc


def _pack_weights(Wih0, Whh0, bih0, bhh0, Wih1, Whh1, bih1, bhh1):
    import ml_dtypes
    bf16 = ml_dtypes.bfloat16
    I8 = np.eye(8, dtype=np.float32)
    w0 = np.zeros((128, 128), np.float32)
    w0[:, 0:64] = Wih0.T[0:128]
    w0[:, 64:128] = Wih0.T[128:256]
    w0h = w0.astype(bf16)
    w0l = (w0 - w0h.astype(np.float32)).astype(bf16)

    def blk(W):  # (64,16) -> (128,512); tau slice = kron(I8, W_tau^T)
        out = np.zeros((128, 512), np.float32)
        for tau in range(4):
            out[:, tau * 128:(tau + 1) * 128] = np.kron(
                I8, W[tau * 16:(tau + 1) * 16].T)
        return out

    def bv(bi, bh):
        b = (bi + bh).astype(np.float32)
        out = np.zeros((128, 4), np.float32)
        for tau in range(4):
            out[:, tau] = np.tile(b[tau * 16:(tau + 1) * 16], 8)
        return out

    return {
        "w0h": w0h,
        "w0l": w0l,
        "fb0": blk(Whh0),
        "xw1": blk(Wih1),
        "fb1": blk(Whh1),
        "ident": np.eye(128, dtype=np.float32),
        "b0": bv(bih0, bhh0),
        "b1": bv(bih1, bhh1),
        "mp": np.kron(I8, np.ones((16, 1), np.float32) / 16.0),
    }


def kernel(x, Wih0, Whh0, bih0, bhh0, Wih1, Whh1, bih1, bhh1, _trace=False):
    from concourse import bass_utils

    import ml_dtypes
    x = np.asarray(x, dtype=np.float32)
    x_hi = x.astype(ml_dtypes.bfloat16)
    x_lo = (x - x_hi.astype(np.float32)).astype(ml_dtypes.bfloat16)
    consts = _pack_weights(
        np.asarray(Wih0, np.float32), np.asarray(Whh0, np.float32),
        np.asarray(bih0, np.float32), np.asarray(bhh0, np.float32),
        np.asarray(Wih1, np.float32), np.asarray(Whh1, np.float32),
        np.asarray(bih1, np.float32), np.asarray(bhh1, np.float32))

    if "nc" not in _cache:
        _cache["nc"] = _build_module()
    nc = _cache["nc"]

    in_maps = []
    for c in range(N_CORES):
        m = {"xh": np.ascontiguousarray(x_hi[c * B_LOC:(c + 1) * B_LOC]),
             "xl": np.ascontiguousarray(x_lo[c * B_LOC:(c + 1) * B_LOC])}
        m.update(consts)
        in_maps.append(m)

    res = bass_utils.run_bass_kernel_spmd(
        nc, in_maps, core_ids=list(range(N_CORES)), trace=_trace)
    y = np.concatenate([r["y"] for r in res.results], axis=0)
    if _trace:
        _cache["last_results"] = res
    return y



# revision 31
# speedup vs baseline: 13153.8589x; 1.0067x over previous
"""Two-layer LSTM (batch=64, feature=256, seq=2048, hidden=16) + mean-pool
over hidden dim, on 8 Trainium2 NeuronCores.

Strategy
--------
Data-parallel over batch: each core gets 8 samples. Per core, the LSTM
recurrence is solved by Picard (Jacobi) iteration over the whole sequence
instead of a 2048-step serial loop: the cell-state propagation
c_t = sigmoid(f_t)*c_{t-1} + z_t is computed exactly each sweep with the
DVE tensor_tensor_scan instruction, and only the small h-feedback term
(Whh @ h_{t-1}, spectral gain ~0.16 for layer 0 / ~0.3 for layer 1) is
lagged one sweep. Each sweep is fully parallel over time, so it runs as a
handful of [128, 2048] tile ops. 5 / 7 sweeps converge to ~2e-4 relative
(the floor set by fp32r matmul rounding; more sweeps reach ~1.6e-4).

Layout: partition p = b_local*16 + j (8 samples x 16 hidden channels = 128
partitions), free dim = time. Gate pre-activations are produced per gate
type tau in {i,f,g,o} as [128, 2048] tiles; the h-feedback GEMM uses a
block-diagonal lhsT = kron(I8, Whh_tau^T) so one K=128 matmul covers all 8
samples. Layer 1 needs no materialized input GEMM: its per-sweep "input"
matmul is blockdiag(Wih1) @ h0 directly. The layer-0 input GEMM runs in
natural [64-gates, time] layout per sample pair and is re-laid-out to
[(b,j), tau*T+t] with 16-partition SBUF->SBUF DMAs on the GPSIMD DGE
queue (so it doesn't stall the x input stream on the SP queue).

Precision: sweep matmuls use fp32r (1 PE cycle/row vs 4 for fp32; TF32-ish
~1.6e-4 operand rounding, verified on HW). The layer-0 input GEMM uses a
host-side bf16 hi/lo split of x and Wih0 with three accumulated bf16
matmuls (~2e-5 accurate, and bf16 matmuls have no fp32r dst-partition-0
restriction, which the per-pair [128=2 samples x 64 gates, T] psum layout
would violate). Gate biases ride the ACT activation bias port.
"""

import numpy as np
from contextlib import ExitStack

B_TOT, D_IN, T_SEQ, H_DIM = 64, 256, 2048, 16
N_CORES = 8
B_LOC = B_TOT // N_CORES  # 8
G4 = 4 * H_DIM  # 64
K0_ITERS = 5
K1_ITERS = 7
NSUB = T_SEQ // 512  # matmul free-dim subtiles
USE_F32R = True  # fp32 "replicated" matmul mode: 1 cycle/row vs 4 for fp32
CHUNK_TAIL = 2  # split z/scan/tanh(c)/h ops into this many time chunks
H_ON_POOL = False  # run the h = sig_o * tanh(c) multiply on GPSIMD
TAU_ORDER = (0, 2, 1, 3)  # ACT processing order of gates i,g,f,o
PSUM_HALF = True  # psum tiles [128,1024] bufs=4 + per-half sigma
ACT_BUFS = 1

_cache = {}


def _build_module():
    import concourse.bacc as bacc
    import concourse.mybir as mybir
    import concourse.tile as tile

    f32 = mybir.dt.float32
    f32r = mybir.dt.float32r if USE_F32R else f32
    AF = mybir.ActivationFunctionType
    ALU = mybir.AluOpType

    nc = bacc.Bacc("TRN2", target_bir_lowering=False, debug=False,
                   num_devices=N_CORES)

    def mm(out, lhsT, rhs, start, stop):
        nc.tensor.matmul(out, lhsT, rhs, start=start, stop=stop)

    bf16 = mybir.dt.bfloat16
    xh_d = nc.dram_tensor("xh", (B_LOC, D_IN, T_SEQ), bf16,
                          kind="ExternalInput")
    xl_d = nc.dram_tensor("xl", (B_LOC, D_IN, T_SEQ), bf16,
                          kind="ExternalInput")
    w0h_d = nc.dram_tensor("w0h", (128, 128), bf16, kind="ExternalInput")
    w0l_d = nc.dram_tensor("w0l", (128, 128), bf16, kind="ExternalInput")
    fb0_d = nc.dram_tensor("fb0", (128, 512), f32r, kind="ExternalInput")
    xw1_d = nc.dram_tensor("xw1", (128, 512), f32r, kind="ExternalInput")
    fb1_d = nc.dram_tensor("fb1", (128, 512), f32r, kind="ExternalInput")
    id_d = nc.dram_tensor("ident", (128, 128), f32r, kind="ExternalInput")
    b0_d = nc.dram_tensor("b0", (128, 4), f32, kind="ExternalInput")
    b1_d = nc.dram_tensor("b1", (128, 4), f32, kind="ExternalInput")
    mp_d = nc.dram_tensor("mp", (128, 8), f32r, kind="ExternalInput")
    y_d = nc.dram_tensor("y", (B_LOC, T_SEQ), f32, kind="ExternalOutput")

    with tile.TileContext(nc) as tc, ExitStack() as ctx:
        cpool = ctx.enter_context(tc.tile_pool(name="consts", bufs=1))
        xpool = ctx.enter_context(tc.tile_pool(name="xstage", bufs=12))
        spool = ctx.enter_context(tc.tile_pool(name="stage", bufs=3))
        gpool = ctx.enter_context(tc.tile_pool(name="bigbufs", bufs=1))
        wpool = ctx.enter_context(tc.tile_pool(name="work", bufs=1))
        ppool = ctx.enter_context(
            tc.tile_pool(name="psum", bufs=2, space="PSUM"))

        def const(name, dram):
            t = cpool.tile(list(dram.shape), dram.dtype, name=name)
            nc.sync.dma_start(t[:], dram.ap())
            return t

        w0h_sb = const("w0h_sb", w0h_d)
        w0l_sb = const("w0l_sb", w0l_d)
        fb0_sb = const("fb0_sb", fb0_d)
        xw1_sb = const("xw1_sb", xw1_d)
        fb1_sb = const("fb1_sb", fb1_d)
        id_sb = const("id_sb", id_d)
        b0_sb = const("b0_sb", b0_d)
        b1_sb = const("b1_sb", b1_d)
        mp_sb = const("mp_sb", mp_d)

        xg = gpool.tile([128, 4 * T_SEQ], f32r, name="xg")
        h0 = gpool.tile([128, T_SEQ + 1], f32r, name="h0")
        h1 = gpool.tile([128, T_SEQ + 1], f32r, name="h1")

        # ---- Phase A: layer-0 input GEMM + relayout --------------------
        # Per sample pair: gates psum [128=(2 samples x 64 gates), T].
        # x streamed per (pair, k-chunk) at 2MB grain so the DMA queue
        # stays saturated while PE/copies trail behind.
        for pair in range(4):
            xtiles = {}
            for bl in range(2):
                for k in range(2):
                    b = pair * 2 + bl
                    xth = xpool.tile([128, T_SEQ], bf16, name=f"xh{bl}{k}",
                                     tag="xst")
                    nc.sync.dma_start(xth[:],
                                      xh_d.ap()[b, k * 128:(k + 1) * 128, :])
                    xtl = xpool.tile([128, T_SEQ], bf16, name=f"xl{bl}{k}",
                                     tag="xst")
                    nc.sync.dma_start(xtl[:],
                                      xl_d.ap()[b, k * 128:(k + 1) * 128, :])
                    xtiles[(bl, k)] = (xth, xtl)
            st = spool.tile([128, T_SEQ], f32r, name="st", tag="stage")
            halves = 2 if PSUM_HALF else 1
            HWA = T_SEQ // halves
            for hh in range(halves):
                if PSUM_HALF:
                    ps = ppool.tile([128, HWA], f32, name="psA",
                                    tag="mmh", bufs=4)
                else:
                    ps = ppool.tile([128, T_SEQ], f32, name="psA", tag="mm")
                for bl in range(2):
                    for nn in range(HWA // 512):
                        n = hh * (HWA // 512) + nn
                        osl = slice(nn * 512, (nn + 1) * 512)
                        isl = slice(n * 512, (n + 1) * 512)
                        out = ps[bl * 64:(bl + 1) * 64, osl]
                        for k in range(2):
                            xth, xtl = xtiles[(bl, k)]
                            wh = w0h_sb[:, k * 64:(k + 1) * 64]
                            wl = w0l_sb[:, k * 64:(k + 1) * 64]
                            mm(out, wh, xth[:, isl], k == 0, False)
                            mm(out, wl, xth[:, isl], False, False)
                            mm(out, wh, xtl[:, isl], False, k == 1)
                dstsl = slice(hh * HWA, (hh + 1) * HWA)
                if pair % 2 == 0:
                    nc.scalar.copy(st[:, dstsl], ps[:])
                else:
                    nc.vector.tensor_copy(st[:, dstsl], ps[:])
            for bl in range(2):
                b = pair * 2 + bl
                for tau in range(4):
                    nc.gpsimd.dma_start(
                        xg[b * 16:(b + 1) * 16, tau * T_SEQ:(tau + 1) * T_SEQ],
                        st[bl * 64 + tau * 16:bl * 64 + (tau + 1) * 16, :])

        # ---- Shared per-layer Picard sweep -----------------------------
        def run_layer(h, in_lhsT, in_rhs, in_off, xg_direct, fb_sb,
                      bias_sb, iters):
            # gate pre-acts per sweep: psum_tau = in_lhsT_tau @ in_rhs
            #                                   + fb_tau @ h (it>0)
            # L0: in_lhsT=identity, in_rhs=xg (tau-blocked columns)
            # L1: in_lhsT=blockdiag(Wih1), in_rhs=h0 (no materialized xg1)
            nc.vector.memset(h[:, 0:1].bitcast(f32), 0.0)
            TAUS = TAU_ORDER  # gate processing order on ACT
            CH = T_SEQ // CHUNK_TAIL
            for it in range(iters):
                acts = {}
                for tau in TAUS:
                    func = AF.Tanh if tau == 2 else AF.Sigmoid
                    dst = wpool.tile([128, T_SEQ], f32, name=f"act{tau}",
                                     tag=f"act{tau}", bufs=ACT_BUFS)
                    if it == 0 and xg_direct:
                        nc.scalar.activation(
                            dst[:],
                            in_rhs[:, tau * T_SEQ:tau * T_SEQ + T_SEQ]
                            .bitcast(f32),
                            func, bias=bias_sb[:, tau:tau + 1])
                        acts[tau] = dst
                        continue
                    halves = 2 if PSUM_HALF else 1
                    HW_ = T_SEQ // halves
                    for hh in range(halves):
                        if PSUM_HALF:
                            ps = ppool.tile([128, HW_], f32, name="psI",
                                            tag="mmh", bufs=4)
                        else:
                            ps = ppool.tile([128, T_SEQ], f32, name="psI",
                                            tag="mm")
                        for nn in range(HW_ // 512):
                            n = hh * (HW_ // 512) + nn
                            osl = slice(nn * 512, (nn + 1) * 512)
                            if xg_direct:
                                rsl = slice(tau * T_SEQ + n * 512,
                                            tau * T_SEQ + (n + 1) * 512)
                                lhsT = in_lhsT
                            else:
                                rsl = slice(in_off + n * 512,
                                            in_off + (n + 1) * 512)
                                lhsT = in_lhsT[:, tau * 128:(tau + 1) * 128]
                            mm(ps[:, osl], lhsT, in_rhs[:, rsl],
                               True, it == 0)
                            if it > 0:
                                mm(ps[:, osl],
                                   fb_sb[:, tau * 128:(tau + 1) * 128],
                                   h[:, n * 512:(n + 1) * 512], False, True)
                        nc.scalar.activation(
                            dst[:, hh * HW_:(hh + 1) * HW_], ps[:], func,
                            bias=bias_sb[:, tau:tau + 1])
                    acts[tau] = dst
                cs_prev = None
                for n in range(CHUNK_TAIL):
                    sl = slice(n * CH, (n + 1) * CH)
                    z = wpool.tile([128, CH], f32, name="z", tag="z", bufs=2)
                    cs = wpool.tile([128, CH], f32, name="cs", tag="cs",
                                    bufs=2)
                    tct = wpool.tile([128, CH], f32, name="tct", tag="tct",
                                     bufs=2)
                    nc.vector.tensor_mul(z[:], acts[0][:, sl],
                                         acts[2][:, sl])
                    init = 0.0 if n == 0 else cs_prev[:, CH - 1:CH]
                    nc.vector.tensor_tensor_scan(cs[:], acts[1][:, sl],
                                                 z[:], init,
                                                 ALU.mult, ALU.add)
                    nc.scalar.activation(tct[:], cs[:], AF.Tanh)
                    eng = nc.gpsimd if H_ON_POOL else nc.vector
                    eng.tensor_mul(h[:, 1 + n * CH:1 + (n + 1) * CH],
                                   acts[3][:, sl], tct[:])
                    cs_prev = cs

        # ---- Phase B: layer-0 sweeps -----------------------------------
        run_layer(h0, id_sb, xg, 0, True, fb0_sb, b0_sb, K0_ITERS)

        # ---- Phase D: layer-1 sweeps -----------------------------------
        run_layer(h1, xw1_sb, h0, 1, False, fb1_sb, b1_sb, K1_ITERS)

        # ---- Phase E: mean over hidden dim + store ---------------------
        yt = spool.tile([8, T_SEQ], f32, name="yt", tag="stage")
        halves = 2 if PSUM_HALF else 1
        HWE = T_SEQ // halves
        for hh in range(halves):
            if PSUM_HALF:
                psE = ppool.tile([8, HWE], f32, name="psE", tag="mmh", bufs=4)
            else:
                psE = ppool.tile([8, T_SEQ], f32, name="psE", tag="mm")
            for nn in range(HWE // 512):
                n = hh * (HWE // 512) + nn
                mm(psE[:, nn * 512:(nn + 1) * 512], mp_sb[:],
                   h1[:, 1 + n * 512:1 + (n + 1) * 512], True, True)
            nc.scalar.copy(yt[:, hh * HWE:(hh + 1) * HWE], psE[:])
        nc.sync.dma_start(y_d.ap(), yt[:])

    nc.compile()
    return nc


def _pack_weights(Wih0, Whh0, bih0, bhh0, Wih1, Whh1, bih1, bhh1):
    import ml_dtypes
    bf16 = ml_dtypes.bfloat16
    I8 = np.eye(8, dtype=np.float32)
    w0 = np.zeros((128, 128), np.float32)
    w0[:, 0:64] = Wih0.T[0:128]
    w0[:, 64:128] = Wih0.T[128:256]
    w0h = w0.astype(bf16)
    w0l = (w0 - w0h.astype(np.float32)).astype(bf16)

    def blk(W):  # (64,16) -> (128,512); tau slice = kron(I8, W_tau^T)
        out = np.zeros((128, 512), np.float32)
        for tau in range(4):
            out[:, tau * 128:(tau + 1) * 128] = np.kron(
                I8, W[tau * 16:(tau + 1) * 16].T)
        return out

    def bv(bi, bh):
        b = (bi + bh).astype(np.float32)
        out = np.zeros((128, 4), np.float32)
        for tau in range(4):
            out[:, tau] = np.tile(b[tau * 16:(tau + 1) * 16], 8)
        return out

    return {
        "w0h": w0h,
        "w0l": w0l,
        "fb0": blk(Whh0),
        "xw1": blk(Wih1),
        "fb1": blk(Whh1),
        "ident": np.eye(128, dtype=np.float32),
        "b0": bv(bih0, bhh0),
        "b1": bv(bih1, bhh1),
        "mp": np.kron(I8, np.ones((16, 1), np.float32) / 16.0),
    }


def kernel(x, Wih0, Whh0, bih0, bhh0, Wih1, Whh1, bih1, bhh1, _trace=False):
    from concourse import bass_utils

    import ml_dtypes
    x = np.asarray(x, dtype=np.float32)
    x_hi = x.astype(ml_dtypes.bfloat16)
    x_lo = (x - x_hi.astype(np.float32)).astype(ml_dtypes.bfloat16)
    consts = _pack_weights(
        np.asarray(Wih0, np.float32), np.asarray(Whh0, np.float32),
        np.asarray(bih0, np.float32), np.asarray(bhh0, np.float32),
        np.asarray(Wih1, np.float32), np.asarray(Whh1, np.float32),
        np.asarray(bih1, np.float32), np.asarray(bhh1, np.float32))

    if "nc" not in _cache:
        _cache["nc"] = _build_module()
    nc = _cache["nc"]

    in_maps = []
    for c in range(N_CORES):
        m = {"xh": np.ascontiguousarray(x_hi[c * B_LOC:(c + 1) * B_LOC]),
             "xl": np.ascontiguousarray(x_lo[c * B_LOC:(c + 1) * B_LOC])}
        m.update(consts)
        in_maps.append(m)

    res = bass_utils.run_bass_kernel_spmd(
        nc, in_maps, core_ids=list(range(N_CORES)), trace=_trace)
    y = np.concatenate([r["y"] for r in res.results], axis=0)
    if _trace:
        _cache["last_results"] = res
    return y



# revision 32
# speedup vs baseline: 13182.4709x; 1.0022x over previous
"""Two-layer LSTM (batch=64, feature=256, seq=2048, hidden=16) + mean-pool
over hidden dim, on 8 Trainium2 NeuronCores.

Strategy
--------
Data-parallel over batch: each core gets 8 samples. Per core, the LSTM
recurrence is solved by Picard (Jacobi) iteration over the whole sequence
instead of a 2048-step serial loop: the cell-state propagation
c_t = sigmoid(f_t)*c_{t-1} + z_t is computed exactly each sweep with the
DVE tensor_tensor_scan instruction, and only the small h-feedback term
(Whh @ h_{t-1}, spectral gain ~0.16 for layer 0 / ~0.3 for layer 1) is
lagged one sweep. Each sweep is fully parallel over time, so it runs as a
handful of [128, 2048] tile ops. 5 / 7 sweeps converge to ~2e-4 relative
(the floor set by fp32r matmul rounding; more sweeps reach ~1.6e-4).

Layout: partition p = b_local*16 + j (8 samples x 16 hidden channels = 128
partitions), free dim = time. Gate pre-activations are produced per gate
type tau in {i,f,g,o} as [128, 2048] tiles; the h-feedback GEMM uses a
block-diagonal lhsT = kron(I8, Whh_tau^T) so one K=128 matmul covers all 8
samples. Layer 1 needs no materialized input GEMM: its per-sweep "input"
matmul is blockdiag(Wih1) @ h0 directly. The layer-0 input GEMM runs in
natural [64-gates, time] layout per sample pair and is re-laid-out to
[(b,j), tau*T+t] with 16-partition SBUF->SBUF DMAs on the GPSIMD DGE
queue (so it doesn't stall the x input stream on the SP queue).

Precision: sweep matmuls use fp32r (1 PE cycle/row vs 4 for fp32; TF32-ish
~1.6e-4 operand rounding, verified on HW). The layer-0 input GEMM uses a
host-side bf16 hi/lo split of x and Wih0 with three accumulated bf16
matmuls (~2e-5 accurate, and bf16 matmuls have no fp32r dst-partition-0
restriction, which the per-pair [128=2 samples x 64 gates, T] psum layout
would violate). Gate biases ride the ACT activation bias port.
"""

import numpy as np
from contextlib import ExitStack

B_TOT, D_IN, T_SEQ, H_DIM = 64, 256, 2048, 16
N_CORES = 8
B_LOC = B_TOT // N_CORES  # 8
G4 = 4 * H_DIM  # 64
K0_ITERS = 5
K1_ITERS = 7
NSUB = T_SEQ // 512  # matmul free-dim subtiles
USE_F32R = True  # fp32 "replicated" matmul mode: 1 cycle/row vs 4 for fp32
CHUNK_TAIL = 2  # split z/scan/tanh(c)/h ops into this many time chunks
H_ON_POOL = False  # run the h = sig_o * tanh(c) multiply on GPSIMD
TAU_ORDER = (0, 2, 1, 3)  # ACT processing order of gates i,g,f,o
PSUM_HALF = True  # psum tiles [128,1024] bufs=4 + per-half sigma
ACT_BUFS = 1

_cache = {}


def _build_module():
    import concourse.bacc as bacc
    import concourse.mybir as mybir
    import concourse.tile as tile

    f32 = mybir.dt.float32
    f32r = mybir.dt.float32r if USE_F32R else f32
    AF = mybir.ActivationFunctionType
    ALU = mybir.AluOpType

    nc = bacc.Bacc("TRN2", target_bir_lowering=False, debug=False,
                   num_devices=N_CORES)

    def mm(out, lhsT, rhs, start, stop):
        nc.tensor.matmul(out, lhsT, rhs, start=start, stop=stop)

    bf16 = mybir.dt.bfloat16
    xh_d = nc.dram_tensor("xh", (B_LOC, D_IN, T_SEQ), bf16,
                          kind="ExternalInput")
    xl_d = nc.dram_tensor("xl", (B_LOC, D_IN, T_SEQ), bf16,
                          kind="ExternalInput")
    w0h_d = nc.dram_tensor("w0h", (128, 128), bf16, kind="ExternalInput")
    w0l_d = nc.dram_tensor("w0l", (128, 128), bf16, kind="ExternalInput")
    fb0_d = nc.dram_tensor("fb0", (128, 512), f32r, kind="ExternalInput")
    xw1_d = nc.dram_tensor("xw1", (128, 512), f32r, kind="ExternalInput")
    fb1_d = nc.dram_tensor("fb1", (128, 512), f32r, kind="ExternalInput")
    id_d = nc.dram_tensor("ident", (128, 128), f32r, kind="ExternalInput")
    b0_d = nc.dram_tensor("b0", (128, 4), f32, kind="ExternalInput")
    b1_d = nc.dram_tensor("b1", (128, 4), f32, kind="ExternalInput")
    mp_d = nc.dram_tensor("mp", (128, 8), f32r, kind="ExternalInput")
    y_d = nc.dram_tensor("y", (B_LOC, T_SEQ), f32, kind="ExternalOutput")

    with tile.TileContext(nc) as tc, ExitStack() as ctx:
        cpool = ctx.enter_context(tc.tile_pool(name="consts", bufs=1))
        xpool = ctx.enter_context(tc.tile_pool(name="xstage", bufs=12))
        spool = ctx.enter_context(tc.tile_pool(name="stage", bufs=3))
        gpool = ctx.enter_context(tc.tile_pool(name="bigbufs", bufs=1))
        wpool = ctx.enter_context(tc.tile_pool(name="work", bufs=1))
        ppool = ctx.enter_context(
            tc.tile_pool(name="psum", bufs=2, space="PSUM"))

        def const(name, dram):
            t = cpool.tile(list(dram.shape), dram.dtype, name=name)
            nc.sync.dma_start(t[:], dram.ap())
            return t

        w0h_sb = const("w0h_sb", w0h_d)
        w0l_sb = const("w0l_sb", w0l_d)
        fb0_sb = const("fb0_sb", fb0_d)
        xw1_sb = const("xw1_sb", xw1_d)
        fb1_sb = const("fb1_sb", fb1_d)
        id_sb = const("id_sb", id_d)
        b0_sb = const("b0_sb", b0_d)
        b1_sb = const("b1_sb", b1_d)
        mp_sb = const("mp_sb", mp_d)

        xg = gpool.tile([128, 4 * T_SEQ], f32r, name="xg")
        h0 = gpool.tile([128, T_SEQ + 1], f32r, name="h0")
        h1 = gpool.tile([128, T_SEQ + 1], f32r, name="h1")

        # ---- Phase A: layer-0 input GEMM + relayout --------------------
        # Per sample pair: gates psum [128=(2 samples x 64 gates), T].
        # x streamed per (pair, k-chunk) at 2MB grain so the DMA queue
        # stays saturated while PE/copies trail behind.
        for pair in range(4):
            xtiles = {}
            for bl in range(2):
                for k in range(2):
                    b = pair * 2 + bl
                    xth = xpool.tile([128, T_SEQ], bf16, name=f"xh{bl}{k}",
                                     tag="xst")
                    nc.sync.dma_start(xth[:],
                                      xh_d.ap()[b, k * 128:(k + 1) * 128, :])
                    xtl = xpool.tile([128, T_SEQ], bf16, name=f"xl{bl}{k}",
                                     tag="xst")
                    nc.sync.dma_start(xtl[:],
                                      xl_d.ap()[b, k * 128:(k + 1) * 128, :])
                    xtiles[(bl, k)] = (xth, xtl)
            st = spool.tile([128, T_SEQ], f32r, name="st", tag="stage")
            halves = 2 if PSUM_HALF else 1
            HWA = T_SEQ // halves
            for hh in range(halves):
                if PSUM_HALF:
                    ps = ppool.tile([128, HWA], f32, name="psA",
                                    tag="mmh", bufs=4)
                else:
                    ps = ppool.tile([128, T_SEQ], f32, name="psA", tag="mm")
                for bl in range(2):
                    for nn in range(HWA // 512):
                        n = hh * (HWA // 512) + nn
                        osl = slice(nn * 512, (nn + 1) * 512)
                        isl = slice(n * 512, (n + 1) * 512)
                        out = ps[bl * 64:(bl + 1) * 64, osl]
                        for k in range(2):
                            xth, xtl = xtiles[(bl, k)]
                            wh = w0h_sb[:, k * 64:(k + 1) * 64]
                            wl = w0l_sb[:, k * 64:(k + 1) * 64]
                            mm(out, wh, xth[:, isl], k == 0, False)
                            mm(out, wl, xth[:, isl], False, False)
                            mm(out, wh, xtl[:, isl], False, k == 1)
                dstsl = slice(hh * HWA, (hh + 1) * HWA)
                nc.vector.tensor_copy(st[:, dstsl], ps[:])
            for bl in range(2):
                b = pair * 2 + bl
                for tau in range(4):
                    nc.gpsimd.dma_start(
                        xg[b * 16:(b + 1) * 16, tau * T_SEQ:(tau + 1) * T_SEQ],
                        st[bl * 64 + tau * 16:bl * 64 + (tau + 1) * 16, :])

        # ---- Shared per-layer Picard sweep -----------------------------
        def run_layer(h, in_lhsT, in_rhs, in_off, xg_direct, fb_sb,
                      bias_sb, iters):
            # gate pre-acts per sweep: psum_tau = in_lhsT_tau @ in_rhs
            #                                   + fb_tau @ h (it>0)
            # L0: in_lhsT=identity, in_rhs=xg (tau-blocked columns)
            # L1: in_lhsT=blockdiag(Wih1), in_rhs=h0 (no materialized xg1)
            nc.vector.memset(h[:, 0:1].bitcast(f32), 0.0)
            TAUS = TAU_ORDER  # gate processing order on ACT
            CH = T_SEQ // CHUNK_TAIL
            for it in range(iters):
                acts = {}
                for tau in TAUS:
                    func = AF.Tanh if tau == 2 else AF.Sigmoid
                    dst = wpool.tile([128, T_SEQ], f32, name=f"act{tau}",
                                     tag=f"act{tau}", bufs=ACT_BUFS)
                    if it == 0 and xg_direct:
                        for hh in range(2):
                            hsl = slice(hh * (T_SEQ // 2),
                                        (hh + 1) * (T_SEQ // 2))
                            nc.scalar.activation(
                                dst[:, hsl],
                                in_rhs[:, tau * T_SEQ + hh * (T_SEQ // 2):
                                       tau * T_SEQ + (hh + 1) * (T_SEQ // 2)]
                                .bitcast(f32),
                                func, bias=bias_sb[:, tau:tau + 1])
                        acts[tau] = dst
                        continue
                    halves = 2 if PSUM_HALF else 1
                    HW_ = T_SEQ // halves
                    for hh in range(halves):
                        if PSUM_HALF:
                            ps = ppool.tile([128, HW_], f32, name="psI",
                                            tag="mmh", bufs=4)
                        else:
                            ps = ppool.tile([128, T_SEQ], f32, name="psI",
                                            tag="mm")
                        for nn in range(HW_ // 512):
                            n = hh * (HW_ // 512) + nn
                            osl = slice(nn * 512, (nn + 1) * 512)
                            if xg_direct:
                                rsl = slice(tau * T_SEQ + n * 512,
                                            tau * T_SEQ + (n + 1) * 512)
                                lhsT = in_lhsT
                            else:
                                rsl = slice(in_off + n * 512,
                                            in_off + (n + 1) * 512)
                                lhsT = in_lhsT[:, tau * 128:(tau + 1) * 128]
                            mm(ps[:, osl], lhsT, in_rhs[:, rsl],
                               True, it == 0)
                            if it > 0:
                                mm(ps[:, osl],
                                   fb_sb[:, tau * 128:(tau + 1) * 128],
                                   h[:, n * 512:(n + 1) * 512], False, True)
                        nc.scalar.activation(
                            dst[:, hh * HW_:(hh + 1) * HW_], ps[:], func,
                            bias=bias_sb[:, tau:tau + 1])
                    acts[tau] = dst
                cs_prev = None
                for n in range(CHUNK_TAIL):
                    sl = slice(n * CH, (n + 1) * CH)
                    z = wpool.tile([128, CH], f32, name="z", tag="z", bufs=3)
                    cs = wpool.tile([128, CH], f32, name="cs", tag="cs",
                                    bufs=3)
                    tct = wpool.tile([128, CH], f32, name="tct", tag="tct",
                                     bufs=3)
                    nc.vector.tensor_mul(z[:], acts[0][:, sl],
                                         acts[2][:, sl])
                    init = 0.0 if n == 0 else cs_prev[:, CH - 1:CH]
                    nc.vector.tensor_tensor_scan(cs[:], acts[1][:, sl],
                                                 z[:], init,
                                                 ALU.mult, ALU.add)
                    nc.scalar.activation(tct[:], cs[:], AF.Tanh)
                    eng = nc.gpsimd if H_ON_POOL else nc.vector
                    eng.tensor_mul(h[:, 1 + n * CH:1 + (n + 1) * CH],
                                   acts[3][:, sl], tct[:])
                    cs_prev = cs

        # ---- Phase B: layer-0 sweeps -----------------------------------
        run_layer(h0, id_sb, xg, 0, True, fb0_sb, b0_sb, K0_ITERS)

        # ---- Phase D: layer-1 sweeps -----------------------------------
        run_layer(h1, xw1_sb, h0, 1, False, fb1_sb, b1_sb, K1_ITERS)

        # ---- Phase E: mean over hidden dim + store ---------------------
        yt = spool.tile([8, T_SEQ], f32, name="yt", tag="stage")
        halves = 2 if PSUM_HALF else 1
        HWE = T_SEQ // halves
        for hh in range(halves):
            if PSUM_HALF:
                psE = ppool.tile([8, HWE], f32, name="psE", tag="mmh", bufs=4)
            else:
                psE = ppool.tile([8, T_SEQ], f32, name="psE", tag="mm")
            for nn in range(HWE // 512):
                n = hh * (HWE // 512) + nn
                mm(psE[:, nn * 512:(nn + 1) * 512], mp_sb[:],
                   h1[:, 1 + n * 512:1 + (n + 1) * 512], True, True)
            nc.scalar.copy(yt[:, hh * HWE:(hh + 1) * HWE], psE[:])
        nc.sync.dma_start(y_d.ap(), yt[:])

    nc.compile()
    return nc


def _pack_weights(Wih0, Whh0, bih0, bhh0, Wih1, Whh1, bih1, bhh1):
    import ml_dtypes
    bf16 = ml_dtypes.bfloat16
    I8 = np.eye(8, dtype=np.float32)
    w0 = np.zeros((128, 128), np.float32)
    w0[:, 0:64] = Wih0.T[0:128]
    w0[:, 64:128] = Wih0.T[128:256]
    w0h = w0.astype(bf16)
    w0l = (w0 - w0h.astype(np.float32)).astype(bf16)

    def blk(W):  # (64,16) -> (128,512); tau slice = kron(I8, W_tau^T)
        out = np.zeros((128, 512), np.float32)
        for tau in range(4):
            out[:, tau * 128:(tau + 1) * 128] = np.kron(
                I8, W[tau * 16:(tau + 1) * 16].T)
        return out

    def bv(bi, bh):
        b = (bi + bh).astype(np.float32)
        out = np.zeros((128, 4), np.float32)
        for tau in range(4):
            out[:, tau] = np.tile(b[tau * 16:(tau + 1) * 16], 8)
        return out

    return {
        "w0h": w0h,
        "w0l": w0l,
        "fb0": blk(Whh0),
        "xw1": blk(Wih1),
        "fb1": blk(Whh1),
        "ident": np.eye(128, dtype=np.float32),
        "b0": bv(bih0, bhh0),
        "b1": bv(bih1, bhh1),
        "mp": np.kron(I8, np.ones((16, 1), np.float32) / 16.0),
    }


def kernel(x, Wih0, Whh0, bih0, bhh0, Wih1, Whh1, bih1, bhh1, _trace=False):
    from concourse import bass_utils

    import ml_dtypes
    x = np.asarray(x, dtype=np.float32)
    x_hi = x.astype(ml_dtypes.bfloat16)
    x_lo = (x - x_hi.astype(np.float32)).astype(ml_dtypes.bfloat16)
    consts = _pack_weights(
        np.asarray(Wih0, np.float32), np.asarray(Whh0, np.float32),
        np.asarray(bih0, np.float32), np.asarray(bhh0, np.float32),
        np.asarray(Wih1, np.float32), np.asarray(Whh1, np.float32),
        np.asarray(bih1, np.float32), np.asarray(bhh1, np.float32))

    if "nc" not in _cache:
        _cache["nc"] = _build_module()
    nc = _cache["nc"]

    in_maps = []
    for c in range(N_CORES):
        m = {"xh": np.ascontiguousarray(x_hi[c * B_LOC:(c + 1) * B_LOC]),
             "xl": np.ascontiguousarray(x_lo[c * B_LOC:(c + 1) * B_LOC])}
        m.update(consts)
        in_maps.append(m)

    res = bass_utils.run_bass_kernel_spmd(
        nc, in_maps, core_ids=list(range(N_CORES)), trace=_trace)
    y = np.concatenate([r["y"] for r in res.results], axis=0)
    if _trace:
        _cache["last_results"] = res
    return y



# revision 35
# speedup vs baseline: 13399.4480x; 1.0165x over previous
"""Two-layer LSTM (batch=64, feature=256, seq=2048, hidden=16) + mean-pool
over hidden dim, on 8 Trainium2 NeuronCores.

Strategy
--------
Data-parallel over batch: each core gets 8 samples. Per core, the LSTM
recurrence is solved by Picard (Jacobi) iteration over the whole sequence
instead of a 2048-step serial loop: the cell-state propagation
c_t = sigmoid(f_t)*c_{t-1} + z_t is computed exactly each sweep with the
DVE tensor_tensor_scan instruction, and only the small h-feedback term
(Whh @ h_{t-1}, spectral gain ~0.16 for layer 0 / ~0.3 for layer 1) is
lagged one sweep. Each sweep is fully parallel over time, so it runs as a
handful of [128, 2048] tile ops. 5 / 7 sweeps converge to ~2e-4 relative
(the floor set by fp32r matmul rounding; more sweeps reach ~1.6e-4).

Layout: partition p = b_local*16 + j (8 samples x 16 hidden channels = 128
partitions), free dim = time. Gate pre-activations are produced per gate
type tau in {i,f,g,o} as [128, 2048] tiles; the h-feedback GEMM uses a
block-diagonal lhsT = kron(I8, Whh_tau^T) so one K=128 matmul covers all 8
samples. Layer 1 needs no materialized input GEMM: its per-sweep "input"
matmul is blockdiag(Wih1) @ h0 directly. The layer-0 input GEMM runs in
natural [64-gates, time] layout per sample pair and is re-laid-out to
[(b,j), tau*T+t] with 16-partition SBUF->SBUF DMAs on the GPSIMD DGE
queue (so it doesn't stall the x input stream on the SP queue).

Precision: sweep matmuls use fp32r (1 PE cycle/row vs 4 for fp32; TF32-ish
~1.6e-4 operand rounding, verified on HW). The layer-0 input GEMM uses a
host-side bf16 hi/lo split of x and Wih0 with three accumulated bf16
matmuls (~2e-5 accurate, and bf16 matmuls have no fp32r dst-partition-0
restriction, which the per-pair [128=2 samples x 64 gates, T] psum layout
would violate). Gate biases ride the ACT activation bias port.
"""

import numpy as np
from contextlib import ExitStack

B_TOT, D_IN, T_SEQ, H_DIM = 64, 256, 2048, 16
N_CORES = 8
B_LOC = B_TOT // N_CORES  # 8
G4 = 4 * H_DIM  # 64
K0_ITERS = 5
K1_ITERS = 7
NSUB = T_SEQ // 512  # matmul free-dim subtiles
USE_F32R = True  # fp32 "replicated" matmul mode: 1 cycle/row vs 4 for fp32
CHUNK_TAIL = 2  # split z/scan/tanh(c)/h ops into this many time chunks
H_ON_POOL = False  # run the h = sig_o * tanh(c) multiply on GPSIMD
TAU_ORDER = (0, 2, 1, 3)  # ACT processing order of gates i,g,f,o
PSUM_HALF = True  # psum tiles [128,1024] bufs=4 + per-half sigma
ACT_BUFS = 1

_cache = {}


def _build_module():
    import concourse.bacc as bacc
    import concourse.mybir as mybir
    import concourse.tile as tile

    f32 = mybir.dt.float32
    f32r = mybir.dt.float32r if USE_F32R else f32
    AF = mybir.ActivationFunctionType
    ALU = mybir.AluOpType

    nc = bacc.Bacc("TRN2", target_bir_lowering=False, debug=False,
                   num_devices=N_CORES)

    def mm(out, lhsT, rhs, start, stop):
        nc.tensor.matmul(out, lhsT, rhs, start=start, stop=stop)

    bf16 = mybir.dt.bfloat16
    xh_d = nc.dram_tensor("xh", (B_LOC, D_IN, T_SEQ), bf16,
                          kind="ExternalInput")
    xl_d = nc.dram_tensor("xl", (B_LOC, D_IN, T_SEQ), bf16,
                          kind="ExternalInput")
    w0h_d = nc.dram_tensor("w0h", (128, 128), bf16, kind="ExternalInput")
    w0l_d = nc.dram_tensor("w0l", (128, 128), bf16, kind="ExternalInput")
    fb0_d = nc.dram_tensor("fb0", (128, 512), f32r, kind="ExternalInput")
    xw1_d = nc.dram_tensor("xw1", (128, 512), f32r, kind="ExternalInput")
    fb1_d = nc.dram_tensor("fb1", (128, 512), f32r, kind="ExternalInput")
    id_d = nc.dram_tensor("ident", (128, 128), f32r, kind="ExternalInput")
    b0_d = nc.dram_tensor("b0", (128, 4), f32, kind="ExternalInput")
    b1_d = nc.dram_tensor("b1", (128, 4), f32, kind="ExternalInput")
    mp_d = nc.dram_tensor("mp", (128, 8), f32r, kind="ExternalInput")
    y_d = nc.dram_tensor("y", (B_LOC, T_SEQ), f32, kind="ExternalOutput")

    with tile.TileContext(nc) as tc, ExitStack() as ctx:
        cpool = ctx.enter_context(tc.tile_pool(name="consts", bufs=1))
        xpool = ctx.enter_context(tc.tile_pool(name="xstage", bufs=12))
        spool = ctx.enter_context(tc.tile_pool(name="stage", bufs=3))
        gpool = ctx.enter_context(tc.tile_pool(name="bigbufs", bufs=1))
        wpool = ctx.enter_context(tc.tile_pool(name="work", bufs=1))
        ppool = ctx.enter_context(
            tc.tile_pool(name="psum", bufs=2, space="PSUM"))

        def const(name, dram):
            t = cpool.tile(list(dram.shape), dram.dtype, name=name)
            nc.sync.dma_start(t[:], dram.ap())
            return t

        w0h_sb = const("w0h_sb", w0h_d)
        w0l_sb = const("w0l_sb", w0l_d)
        fb0_sb = const("fb0_sb", fb0_d)
        xw1_sb = const("xw1_sb", xw1_d)
        fb1_sb = const("fb1_sb", fb1_d)
        id_sb = const("id_sb", id_d)
        b0_sb = const("b0_sb", b0_d)
        b1_sb = const("b1_sb", b1_d)
        mp_sb = const("mp_sb", mp_d)

        xg = gpool.tile([128, 4 * T_SEQ], f32r, name="xg")
        h0 = gpool.tile([128, T_SEQ + 1], f32r, name="h0")
        h1 = gpool.tile([128, T_SEQ + 1], f32r, name="h1")

        # ---- Phase A: layer-0 input GEMM + relayout --------------------
        # Per sample pair: gates psum [128=(2 samples x 64 gates), T].
        # x streamed per (pair, k-chunk) at 2MB grain so the DMA queue
        # stays saturated while PE/copies trail behind.
        for pair in range(4):
            xtiles = {}
            for bl in range(2):
                for k in range(2):
                    b = pair * 2 + bl
                    xth = xpool.tile([128, T_SEQ], bf16, name=f"xh{bl}{k}",
                                     tag="xst")
                    nc.sync.dma_start(xth[:],
                                      xh_d.ap()[b, k * 128:(k + 1) * 128, :])
                    xtl = xpool.tile([128, T_SEQ], bf16, name=f"xl{bl}{k}",
                                     tag="xst")
                    nc.sync.dma_start(xtl[:],
                                      xl_d.ap()[b, k * 128:(k + 1) * 128, :])
                    xtiles[(bl, k)] = (xth, xtl)
            st = spool.tile([128, T_SEQ], f32r, name="st", tag="stage")
            halves = 2 if PSUM_HALF else 1
            HWA = T_SEQ // halves
            for hh in range(halves):
                if PSUM_HALF:
                    ps = ppool.tile([128, HWA], f32, name="psA",
                                    tag="mmh", bufs=4)
                else:
                    ps = ppool.tile([128, T_SEQ], f32, name="psA", tag="mm")
                for bl in range(2):
                    for nn in range(HWA // 512):
                        n = hh * (HWA // 512) + nn
                        osl = slice(nn * 512, (nn + 1) * 512)
                        isl = slice(n * 512, (n + 1) * 512)
                        out = ps[bl * 64:(bl + 1) * 64, osl]
                        for k in range(2):
                            xth, xtl = xtiles[(bl, k)]
                            wh = w0h_sb[:, k * 64:(k + 1) * 64]
                            wl = w0l_sb[:, k * 64:(k + 1) * 64]
                            mm(out, wh, xth[:, isl], k == 0, False)
                            mm(out, wl, xth[:, isl], False, False)
                            mm(out, wh, xtl[:, isl], False, k == 1)
                dstsl = slice(hh * HWA, (hh + 1) * HWA)
                nc.vector.tensor_copy(st[:, dstsl], ps[:])
            for bl in range(2):
                b = pair * 2 + bl
                for tau in range(4):
                    nc.gpsimd.dma_start(
                        xg[b * 16:(b + 1) * 16, tau * T_SEQ:(tau + 1) * T_SEQ],
                        st[bl * 64 + tau * 16:bl * 64 + (tau + 1) * 16, :])

        # ---- Shared per-layer Picard sweep -----------------------------
        def run_layer(h, in_lhsT, in_rhs, in_off, xg_direct, fb_sb,
                      bias_sb, iters):
            # gate pre-acts per sweep: psum_tau = in_lhsT_tau @ in_rhs
            #                                   + fb_tau @ h (it>0)
            # L0: in_lhsT=identity, in_rhs=xg (tau-blocked columns)
            # L1: in_lhsT=blockdiag(Wih1), in_rhs=h0 (no materialized xg1)
            nc.vector.memset(h[:, 0:1].bitcast(f32), 0.0)
            TAUS = TAU_ORDER  # gate processing order on ACT
            CH = T_SEQ // CHUNK_TAIL
            for it in range(iters):
                acts = {}
                for tau in TAUS:
                    func = AF.Tanh if tau == 2 else AF.Sigmoid
                    dst = wpool.tile([128, T_SEQ], f32, name=f"act{tau}",
                                     tag=f"act{tau}", bufs=ACT_BUFS)
                    if it == 0 and xg_direct:
                        for hh in range(2):
                            hsl = slice(hh * (T_SEQ // 2),
                                        (hh + 1) * (T_SEQ // 2))
                            nc.scalar.activation(
                                dst[:, hsl],
                                in_rhs[:, tau * T_SEQ + hh * (T_SEQ // 2):
                                       tau * T_SEQ + (hh + 1) * (T_SEQ // 2)]
                                .bitcast(f32),
                                func, bias=bias_sb[:, tau:tau + 1])
                        acts[tau] = dst
                        continue
                    halves = 2 if PSUM_HALF else 1
                    HW_ = T_SEQ // halves
                    for hh in range(halves):
                        if PSUM_HALF:
                            ps = ppool.tile([128, HW_], f32, name="psI",
                                            tag="mmh", bufs=4)
                        else:
                            ps = ppool.tile([128, T_SEQ], f32, name="psI",
                                            tag="mm")
                        # input matmuls first (h-independent), then the
                        # feedback matmuls: PE is in-order, so this lets the
                        # input half run during the previous sweep's tail.
                        for nn in range(HW_ // 512):
                            n = hh * (HW_ // 512) + nn
                            osl = slice(nn * 512, (nn + 1) * 512)
                            if xg_direct:
                                rsl = slice(tau * T_SEQ + n * 512,
                                            tau * T_SEQ + (n + 1) * 512)
                                lhsT = in_lhsT
                            else:
                                rsl = slice(in_off + n * 512,
                                            in_off + (n + 1) * 512)
                                lhsT = in_lhsT[:, tau * 128:(tau + 1) * 128]
                            mm(ps[:, osl], lhsT, in_rhs[:, rsl],
                               True, it == 0)
                        if it > 0:
                            for nn in range(HW_ // 512):
                                n = hh * (HW_ // 512) + nn
                                osl = slice(nn * 512, (nn + 1) * 512)
                                mm(ps[:, osl],
                                   fb_sb[:, tau * 128:(tau + 1) * 128],
                                   h[:, n * 512:(n + 1) * 512], False, True)
                        nc.scalar.activation(
                            dst[:, hh * HW_:(hh + 1) * HW_], ps[:], func,
                            bias=bias_sb[:, tau:tau + 1])
                    acts[tau] = dst
                cs_prev = None
                for n in range(CHUNK_TAIL):
                    sl = slice(n * CH, (n + 1) * CH)
                    z = wpool.tile([128, CH], f32, name="z", tag="z", bufs=3)
                    cs = wpool.tile([128, CH], f32, name="cs", tag="cs",
                                    bufs=3)
                    tct = wpool.tile([128, CH], f32, name="tct", tag="tct",
                                     bufs=3)
                    nc.vector.tensor_mul(z[:], acts[0][:, sl],
                                         acts[2][:, sl])
                    init = 0.0 if n == 0 else cs_prev[:, CH - 1:CH]
                    nc.vector.tensor_tensor_scan(cs[:], acts[1][:, sl],
                                                 z[:], init,
                                                 ALU.mult, ALU.add)
                    nc.scalar.activation(tct[:], cs[:], AF.Tanh)
                    eng = nc.gpsimd if H_ON_POOL else nc.vector
                    eng.tensor_mul(h[:, 1 + n * CH:1 + (n + 1) * CH],
                                   acts[3][:, sl], tct[:])
                    cs_prev = cs

        # ---- Phase B: layer-0 sweeps -----------------------------------
        run_layer(h0, id_sb, xg, 0, True, fb0_sb, b0_sb, K0_ITERS)

        # ---- Phase D: layer-1 sweeps -----------------------------------
        run_layer(h1, xw1_sb, h0, 1, False, fb1_sb, b1_sb, K1_ITERS)

        # ---- Phase E: mean over hidden dim + store ---------------------
        yt = spool.tile([8, T_SEQ], f32, name="yt", tag="stage")
        halves = 2 if PSUM_HALF else 1
        HWE = T_SEQ // halves
        for hh in range(halves):
            if PSUM_HALF:
                psE = ppool.tile([8, HWE], f32, name="psE", tag="mmh", bufs=4)
            else:
                psE = ppool.tile([8, T_SEQ], f32, name="psE", tag="mm")
            for nn in range(HWE // 512):
                n = hh * (HWE // 512) + nn
                mm(psE[:, nn * 512:(nn + 1) * 512], mp_sb[:],
                   h1[:, 1 + n * 512:1 + (n + 1) * 512], True, True)
            nc.scalar.copy(yt[:, hh * HWE:(hh + 1) * HWE], psE[:])
        nc.sync.dma_start(y_d.ap(), yt[:])

    nc.compile()
    return nc


def _pack_weights(Wih0, Whh0, bih0, bhh0, Wih1, Whh1, bih1, bhh1):
    import ml_dtypes
    bf16 = ml_dtypes.bfloat16
    I8 = np.eye(8, dtype=np.float32)
    w0 = np.zeros((128, 128), np.float32)
    w0[:, 0:64] = Wih0.T[0:128]
    w0[:, 64:128] = Wih0.T[128:256]
    w0h = w0.astype(bf16)
    w0l = (w0 - w0h.astype(np.float32)).astype(bf16)

    def blk(W):  # (64,16) -> (128,512); tau slice = kron(I8, W_tau^T)
        out = np.zeros((128, 512), np.float32)
        for tau in range(4):
            out[:, tau * 128:(tau + 1) * 128] = np.kron(
                I8, W[tau * 16:(tau + 1) * 16].T)
        return out

    def bv(bi, bh):
        b = (bi + bh).astype(np.float32)
        out = np.zeros((128, 4), np.float32)
        for tau in range(4):
            out[:, tau] = np.tile(b[tau * 16:(tau + 1) * 16], 8)
        return out

    return {
        "w0h": w0h,
        "w0l": w0l,
        "fb0": blk(Whh0),
        "xw1": blk(Wih1),
        "fb1": blk(Whh1),
        "ident": np.eye(128, dtype=np.float32),
        "b0": bv(bih0, bhh0),
        "b1": bv(bih1, bhh1),
        "mp": np.kron(I8, np.ones((16, 1), np.float32) / 16.0),
    }


def kernel(x, Wih0, Whh0, bih0, bhh0, Wih1, Whh1, bih1, bhh1, _trace=False):
    from concourse import bass_utils

    import ml_dtypes
    x = np.asarray(x, dtype=np.float32)
    x_hi = x.astype(ml_dtypes.bfloat16)
    x_lo = (x - x_hi.astype(np.float32)).astype(ml_dtypes.bfloat16)
    consts = _pack_weights(
        np.asarray(Wih0, np.float32), np.asarray(Whh0, np.float32),
        np.asarray(bih0, np.float32), np.asarray(bhh0, np.float32),
        np.asarray(Wih1, np.float32), np.asarray(Whh1, np.float32),
        np.asarray(bih1, np.float32), np.asarray(bhh1, np.float32))

    if "nc" not in _cache:
        _cache["nc"] = _build_module()
    nc = _cache["nc"]

    in_maps = []
    for c in range(N_CORES):
        m = {"xh": np.ascontiguousarray(x_hi[c * B_LOC:(c + 1) * B_LOC]),
             "xl": np.ascontiguousarray(x_lo[c * B_LOC:(c + 1) * B_LOC])}
        m.update(consts)
        in_maps.append(m)

    res = bass_utils.run_bass_kernel_spmd(
        nc, in_maps, core_ids=list(range(N_CORES)), trace=_trace)
    y = np.concatenate([r["y"] for r in res.results], axis=0)
    if _trace:
        _cache["last_results"] = res
    return y

